# revision 4
# baseline (speedup 1.0000x reference)
"""Fused multi-head attention forward (B=2, S=2048, SIZE=1024, H=16) on 8
Trainium2 NeuronCores.

Sharding: 2-way data parallel over batch x 4-way tensor parallel over heads
(Megatron style). Each core computes 4 heads of one batch element end-to-end
(QKV projections for its 256-dim slice, attention, and a partial output
projection); the host sums the 4 partials per batch and adds the output
bias. The value-projection bias drops out of attention algebraically
(softmax rows sum to 1), so the host folds `bv @ Wo.T` into that same
constant row.

v2: the attention inner loop is pipelined across three engines so the PE
never waits on softmax:
  - per kt, both heads' score matmuls run as concurrent row-group streams
    (kh pair packed on partitions 0-63 / 64-127);
  - exp is column-split: ScalarE computes exact exp on cols [0:SPLIT) of
    each head's score tile while VectorE computes cols [SPLIT:1024) with a
    one-instruction Schraudolph approximation (a*x+b written as int16,
    bit-aliased as bf16);
  - the softmax denominator is free: vh carries a 65th ones-column, so row
    64 of each [65,512] ctx accumulator is sum(exp) — no reduction ops;
  - ctx PSUM evacuation goes over DMA (PSUM->SBUF, remapping head 1 to
    partitions 64-127), the denominator reciprocal runs on a 64-partition
    spread via the fast DVE custom op, and normalization is one fused
    scalar_tensor_tensor per (head, j).
PSUM: 2 x [128,1024] score tiles (single-buffered per head, heads run
phase-shifted) + 4 x [65,512] ctx accumulators = 8 banks exactly.
"""

import numpy as np
import ml_dtypes

import concourse.bass as bass
import concourse.tile as tile
from concourse import bacc, mybir
from concourse.bass_utils import run_bass_kernel_spmd

B, S, SIZE, H, D = 2, 2048, 1024, 16, 64
NCORES = 8
HGROUPS = 4                # tensor-parallel head groups
H_LOC = H // HGROUPS       # 4 heads per core
D_LOC = H_LOC * D          # 256 projection dims per core
MT = D_LOC // 128          # 2 head-pairs per core
ET = SIZE // 128           # 8 contraction tiles for projections
ST = S // 128              # 16 sequence tiles of 128
KT = S // 128              # 16 key tiles

SPLIT = 624                # exp column split: [0:SPLIT) ScalarE, rest DVE
EXP_A = float(128.0 / np.log(2.0))      # Schraudolph scale (bf16 bits)
EXP_B = float(127.0 * 128.0 - 11.0)     # Schraudolph bias incl. shift C=11

_NC = None


def build():
    global _NC
    if _NC is not None:
        return _NC
    f32, bf16, i16 = mybir.dt.float32, mybir.dt.bfloat16, mybir.dt.int16
    Exp = mybir.ActivationFunctionType.Exp
    mul_op = mybir.AluOpType.mult
    add_op = mybir.AluOpType.add

    nc = bacc.Bacc("TRN2", target_bir_lowering=False, debug=False)
    qT_d = nc.dram_tensor("qT", [SIZE, S], bf16, kind="ExternalInput").ap()
    kT_d = nc.dram_tensor("kT", [SIZE, S], bf16, kind="ExternalInput").ap()
    vT_d = nc.dram_tensor("vT", [SIZE, S], bf16, kind="ExternalInput").ap()
    WqT_d = nc.dram_tensor("WqT", [SIZE, D_LOC], bf16, kind="ExternalInput").ap()
    WkT_d = nc.dram_tensor("WkT", [SIZE, D_LOC], bf16, kind="ExternalInput").ap()
    WvT_d = nc.dram_tensor("WvT", [SIZE, D_LOC], bf16, kind="ExternalInput").ap()
    WoT_d = nc.dram_tensor("WoT", [D_LOC, SIZE], bf16, kind="ExternalInput").ap()
    bq_d = nc.dram_tensor("bq", [D_LOC], f32, kind="ExternalInput").ap()
    bk_d = nc.dram_tensor("bk", [D_LOC], f32, kind="ExternalInput").ap()
    out_d = nc.dram_tensor("out", [S, SIZE], f32, kind="ExternalOutput").ap()

    qTt = qT_d.rearrange("(et p) s -> p et s", p=128)
    kTt = kT_d.rearrange("(et p) s -> p et s", p=128)
    vTt = vT_d.rearrange("(et p) s -> p et s", p=128)

    with tile.TileContext(nc) as tc:
        with tc.tile_pool(name="persist", bufs=1) as persist:
            wq_sb = persist.tile([128, ET, D_LOC], bf16)
            wk_sb = persist.tile([128, ET, D_LOC], bf16)
            wv_sb = persist.tile([128, ET, D_LOC], bf16)
            nc.sync.dma_start(wv_sb[:], WvT_d.rearrange("(et p) m -> p et m", p=128))
            nc.sync.dma_start(wq_sb[:], WqT_d.rearrange("(et p) m -> p et m", p=128))
            nc.sync.dma_start(wk_sb[:], WkT_d.rearrange("(et p) m -> p et m", p=128))
            wo_sb = persist.tile([128, MT, SIZE], bf16)
            nc.sync.dma_start(wo_sb[:], WoT_d.rearrange("(hp p) o -> p hp o", p=128))
            bq_sb = persist.tile([128, MT], f32)
            bk_sb = persist.tile([128, MT], f32)
            nc.sync.dma_start(bq_sb[:], bq_d.rearrange("(mt p) -> p mt", p=128))
            nc.sync.dma_start(bk_sb[:], bk_d.rearrange("(mt p) -> p mt", p=128))

            qh_sb = persist.tile([128, MT, S], bf16)   # [dim within pair, pair, s]
            kh_sb = persist.tile([128, MT, S], bf16)
            # [s%128, s//128, head, d] with a 65th ones-column per head: the
            # ctx matmul then accumulates sum(exp) in accumulator row 64.
            vh_sb = persist.tile([128, ST, H_LOC, D + 1], bf16)
            nc.vector.memset(vh_sb[:, :, :, D:D + 1], 1.0)
            ctxn_sb = persist.tile([128, MT, S], bf16)  # normalized ctxT
            ones_f32 = persist.tile([128, 1], f32)
            nc.vector.memset(ones_f32[:], 1.0)
            ones_bf = persist.tile([128, 1], bf16)
            nc.vector.tensor_copy(ones_bf[:], ones_f32[:])

            # ---------- phase A: projections, one row-DMA per contraction tile ----------
            with (
                tc.tile_pool(name="xinV", bufs=4) as xinV,
                tc.tile_pool(name="psV", bufs=1, space="PSUM") as psV,
            ):
                # V first, in 2 passes of 8 sequence tiles (8 PSUM banks each)
                for vpass in range(2):
                    psv = [psV.tile([128, D_LOC], f32, tag=f"pv{i}", name=f"pv{i}")
                           for i in range(8)]
                    for et in range(ET):
                        vrow = xinV.tile([128, 1024], bf16, tag="vrow")
                        nc.sync.dma_start(
                            vrow[:], vTt[:, et, vpass * 1024:(vpass + 1) * 1024])
                        for i in range(8):
                            nc.tensor.matmul(
                                psv[i][:], vrow[:, i * 128:(i + 1) * 128], wv_sb[:, et, :],
                                start=(et == 0), stop=(et == ET - 1))
                    for i in range(8):
                        st = vpass * 8 + i
                        nc.vector.tensor_copy(
                            vh_sb[:, st, :, 0:D],
                            psv[i].rearrange("p (h d) -> p h d", h=H_LOC))
            with (
                tc.tile_pool(name="xinQK", bufs=4) as xinQK,
                tc.tile_pool(name="psQK", bufs=1, space="PSUM") as psQK,
            ):
                for src, wsb, bsb, dst in (
                    (kTt, wk_sb, bk_sb, kh_sb),
                    (qTt, wq_sb, bq_sb, qh_sb),
                ):
                    pss = [psQK.tile([128, 512], f32, tag=f"pa{i}", name=f"pa{i}")
                           for i in range(8)]
                    for et in range(ET):
                        xrow = xinQK.tile([128, S], bf16, tag="xrow")
                        nc.sync.dma_start(xrow[:, 0:1024], src[:, et, 0:1024])
                        nc.sync.dma_start(xrow[:, 1024:2048], src[:, et, 1024:2048])
                        for nt in range(4):
                            for mt in range(MT):
                                nc.tensor.matmul(
                                    pss[nt * MT + mt][:],
                                    wsb[:, et, mt * 128:(mt + 1) * 128],
                                    xrow[:, nt * 512:(nt + 1) * 512],
                                    start=(et == 0), stop=(et == ET - 1))
                    for nt in range(4):
                        for mt in range(MT):
                            nc.vector.tensor_scalar_add(
                                dst[:, mt, nt * 512:(nt + 1) * 512],
                                pss[nt * MT + mt][:], bsb[:, mt:mt + 1])

            # ---------- phase B: attention ----------
            with (
                tc.tile_pool(name="psS", bufs=1, space="PSUM") as psS,
                tc.tile_pool(name="psC", bufs=1, space="PSUM") as psC,
                tc.tile_pool(name="esb", bufs=2) as esb,
                tc.tile_pool(name="smalls", bufs=2) as smalls,
                tc.tile_pool(name="dscr", bufs=2, space="DRAM") as dscr,
            ):
                for pr in range(MT):
                    for sw in range(2):
                        q0s = sw * 1024
                        scs = [psS.tile([128, 1024], f32, tag=f"sc{hi}",
                                        name=f"sc{hi}")
                               for hi in range(2)]
                        # full-bank ctx accumulators per j:
                        #   cpa[j]: rows 0-64 = h0 ctx + h0 den (M=65 with the
                        #           ones column), row 96 = h1 den (M=1 matmul,
                        #           col-strip 3 -> concurrent with h0's ctx)
                        #   cpb[j]: rows 64-127 = h1 ctx (base-64 slice)
                        cpa = [psC.tile([128, 512], f32, tag=f"ca{j}",
                                        name=f"ca{j}") for j in range(2)]
                        cpb = [psC.tile([128, 512], f32, tag=f"cb{j}",
                                        name=f"cb{j}") for j in range(2)]
                        ebufs = [[esb.tile([128, 1024], bf16, tag=f"e{hi}{a}",
                                           name=f"e{hi}{a}")
                                  for a in range(2)]
                                 for hi in range(2)]
                        for kt in range(KT):
                            for hi in range(2):
                                po = hi * D
                                for j in range(2):
                                    nc.tensor.matmul(
                                        scs[hi][:, j * 512:(j + 1) * 512],
                                        kh_sb[po:po + D, pr, kt * 128:(kt + 1) * 128],
                                        qh_sb[po:po + D, pr,
                                              q0s + j * 512:q0s + (j + 1) * 512],
                                        start=True, stop=True)
                            for hi in range(2):
                                e_sb = ebufs[hi][kt % 2]
                                nc.scalar.activation(
                                    e_sb[:, 0:SPLIT], scs[hi][:, 0:SPLIT], Exp)
                                nc.vector.tensor_scalar(
                                    e_sb[:, SPLIT:1024].bitcast(i16),
                                    scs[hi][:, SPLIT:1024],
                                    EXP_A, EXP_B, mul_op, add_op)
                            st_, sp_ = (kt == 0), (kt == KT - 1)
                            for j in range(2):
                                e0 = ebufs[0][kt % 2][:, j * 512:(j + 1) * 512]
                                e1 = ebufs[1][kt % 2][:, j * 512:(j + 1) * 512]
                                nc.tensor.matmul(
                                    cpa[j][0:D + 1, :],
                                    vh_sb[:, kt, pr * 2, :], e0,
                                    start=st_, stop=sp_)
                                nc.tensor.matmul(
                                    cpa[j][96:97, :], ones_bf[:], e1,
                                    start=st_, stop=sp_,
                                    tile_position=(0, 96))
                                nc.tensor.matmul(
                                    cpb[j][D:128, :],
                                    vh_sb[:, kt, pr * 2 + 1, 0:D], e1,
                                    start=st_, stop=sp_)
                        # evacuate ctx + denominators (lane-aligned DVE copies)
                        cua = smalls.tile([128, 1024], f32, tag="cua")
                        cub = smalls.tile([128, 1024], f32, tag="cub")
                        for j in range(2):
                            nc.vector.tensor_copy(
                                cua[:, j * 512:(j + 1) * 512], cpa[j][:])
                            nc.vector.tensor_copy(
                                cub[D:128, j * 512:(j + 1) * 512],
                                cpb[j][D:128, :])
                        scr = dscr.tile([2048], f32, tag="scr")
                        for j in range(2):
                            nc.sync.dma_start(
                                scr[j * 512:(j + 1) * 512]
                                .rearrange("(a b) -> a b", a=1),
                                cua[D:D + 1, j * 512:(j + 1) * 512])
                            nc.sync.dma_start(
                                scr[(2 + j) * 512:(3 + j) * 512]
                                .rearrange("(a b) -> a b", a=1),
                                cua[96:97, j * 512:(j + 1) * 512])
                        # reciprocal on a 64-partition spread, then broadcast
                        spread = smalls.tile([64, 32], f32, tag="spread")
                        nc.sync.dma_start(
                            spread[:], scr[:].rearrange("(p j) -> p j", p=64))
                        spread_r = smalls.tile([64, 32], f32, tag="spreadr")
                        nc.vector.reciprocal_approx_fast(spread_r[:], spread[:])
                        scr2 = dscr.tile([2048], f32, tag="scr2")
                        nc.sync.dma_start(
                            scr2[:].rearrange("(p j) -> p j", p=64), spread_r[:])
                        brec = smalls.tile([128, 1024], f32, tag="brec")
                        for hi in range(2):
                            for j in range(2):
                                hj = hi * 2 + j
                                part = scr2[hj * 512:(hj + 1) * 512]
                                nc.sync.dma_start(
                                    brec[hi * D:(hi + 1) * D,
                                         j * 512:(j + 1) * 512],
                                    bass.AP(tensor=part.tensor, offset=part.offset,
                                            ap=[[0, D]] + list(part.ap)))
                        # fused normalize: ctxn = (cu + 0) * brec
                        for hi in range(2):
                            rows = slice(hi * D, (hi + 1) * D)
                            cu = cua if hi == 0 else cub
                            for j in range(2):
                                nc.vector.scalar_tensor_tensor(
                                    ctxn_sb[rows, pr,
                                            q0s + j * 512:q0s + (j + 1) * 512],
                                    cu[rows, j * 512:(j + 1) * 512],
                                    0.0,
                                    brec[rows, j * 512:(j + 1) * 512],
                                    add_op, mul_op)

            # ---------- phase C: output projection (partial over local dims) ----------
            with (
                tc.tile_pool(name="psD", bufs=4, space="PSUM") as psD,
                tc.tile_pool(name="osb", bufs=4) as osb,
            ):
                for st in range(ST):
                    for ot in range(SIZE // 512):
                        pso = psD.tile([128, 512], f32, tag="po")
                        for hp in range(MT):
                            nc.tensor.matmul(
                                pso[:],
                                ctxn_sb[:, hp, st * 128:(st + 1) * 128],
                                wo_sb[:, hp, ot * 512:(ot + 1) * 512],
                                start=(hp == 0), stop=(hp == MT - 1))
                        o_sb = osb.tile([128, 512], f32, tag="o")
                        if (st * 2 + ot) % 2 == 0:
                            nc.vector.tensor_copy(o_sb[:], pso[:])
                        else:
                            nc.scalar.copy(o_sb[:], pso[:])
                        nc.sync.dma_start(
                            out_d[st * 128:(st + 1) * 128, ot * 512:(ot + 1) * 512],
                            o_sb[:])

    nc.compile()
    _NC = nc
    return nc


def prepare_in_maps(inputs):
    q, k, v = inputs["q"], inputs["k"], inputs["v"]
    Wq, bq = inputs["Wq"], inputs["bq"]
    Wk, bk = inputs["Wk"], inputs["bk"]
    Wv = inputs["Wv"]
    Wo = inputs["Wo"]
    sc = np.float32(1.0 / np.sqrt(D))

    f32, bf = np.float32, ml_dtypes.bfloat16
    qT = [q[b].T.astype(bf) for b in range(B)]
    kT = [k[b].T.astype(bf) for b in range(B)]
    vT = [v[b].T.astype(bf) for b in range(B)]
    WqTs = (Wq.T * sc).astype(bf)   # scale folded into Wq
    WkT = Wk.T.astype(bf)
    WvT = Wv.T.astype(bf)
    WoT = Wo.T.astype(bf)           # [c, o]
    bqs = (bq * sc).astype(f32)

    in_maps = []
    for core in range(NCORES):
        b, hg = divmod(core, HGROUPS)
        sl = slice(hg * D_LOC, (hg + 1) * D_LOC)
        in_maps.append({
            "qT": qT[b], "kT": kT[b], "vT": vT[b],
            "WqT": np.ascontiguousarray(WqTs[:, sl]),
            "WkT": np.ascontiguousarray(WkT[:, sl]),
            "WvT": np.ascontiguousarray(WvT[:, sl]),
            "WoT": np.ascontiguousarray(WoT[sl, :]),
            "bq": np.ascontiguousarray(bqs[sl]),
            "bk": np.ascontiguousarray(bk[sl].astype(f32)),
        })
    return in_maps


def gather(results, inputs):
    # host epilogue: sum the 4 tensor-parallel partials per batch and add the
    # constant row bv @ Wo.T + bo (the value bias commutes through softmax)
    const = (inputs["bv"].astype(np.float64) @ inputs["Wo"].astype(np.float64).T
             + inputs["bo"].astype(np.float64)).astype(np.float32)
    full = np.empty((B, S, SIZE), np.float32)
    for b in range(B):
        acc = results[b * HGROUPS]["out"].astype(np.float32).copy()
        for hg in range(1, HGROUPS):
            acc += results[b * HGROUPS + hg]["out"]
        full[b] = acc + const[None, :]
    return full


def kernel(**inputs):
    nc = build()
    in_maps = prepare_in_maps(inputs)
    res = run_bass_kernel_spmd(nc, in_maps, core_ids=list(range(NCORES)), trace=False)
    return gather(res.results, inputs)


# revision 8
# speedup vs baseline: 1.1340x; 1.1340x over previous
"""Fused multi-head attention forward (B=2, S=2048, SIZE=1024, H=16) on 8
Trainium2 NeuronCores.

Sharding: 2-way data parallel over batch x 4-way tensor parallel over heads
(Megatron style). Each core computes 4 heads of one batch element end-to-end
(QKV projections for its 256-dim slice, attention, and a partial output
projection); the host sums the 4 partials per batch and adds the output
bias. The value-projection bias drops out of attention algebraically
(softmax rows sum to 1), so the host folds `bv @ Wo.T` into that same
constant row.

v2: the attention inner loop is pipelined across three engines so the PE
never waits on softmax:
  - per kt, both heads' score matmuls run as concurrent row-group streams
    (kh pair packed on partitions 0-63 / 64-127);
  - exp is column-split: ScalarE computes exact exp on cols [0:SPLIT) of
    each head's score tile while VectorE computes cols [SPLIT:1024) with a
    one-instruction Schraudolph approximation (a*x+b written as int16,
    bit-aliased as bf16);
  - the softmax denominator is free: vh carries a 65th ones-column, so row
    64 of each [65,512] ctx accumulator is sum(exp) — no reduction ops;
  - ctx PSUM evacuation goes over DMA (PSUM->SBUF, remapping head 1 to
    partitions 64-127), the denominator reciprocal runs on a 64-partition
    spread via the fast DVE custom op, and normalization is one fused
    scalar_tensor_tensor per (head, j).
PSUM: 2 x [128,1024] score tiles (single-buffered per head, heads run
phase-shifted) + 4 x [65,512] ctx accumulators = 8 banks exactly.
"""

import numpy as np
import ml_dtypes

import concourse.bass as bass
import concourse.tile as tile
from concourse import bacc, mybir
from concourse.bass_utils import run_bass_kernel_spmd

B, S, SIZE, H, D = 2, 2048, 1024, 16, 64
NCORES = 8
HGROUPS = 4                # tensor-parallel head groups
H_LOC = H // HGROUPS       # 4 heads per core
D_LOC = H_LOC * D          # 256 projection dims per core
MT = D_LOC // 128          # 2 head-pairs per core
ET = SIZE // 128           # 8 contraction tiles for projections
ST = S // 128              # 16 sequence tiles of 128
KT = S // 128              # 16 key tiles

SPLIT = 624                # exp column split: [0:SPLIT) ScalarE, rest DVE
EXP_A = float(128.0 / np.log(2.0))      # Schraudolph scale (bf16 bits)
EXP_B = float(127.0 * 128.0 - 11.0)     # Schraudolph bias incl. shift C=11

_NC = None


def build():
    global _NC
    if _NC is not None:
        return _NC
    f32, bf16, i16 = mybir.dt.float32, mybir.dt.bfloat16, mybir.dt.int16
    Exp = mybir.ActivationFunctionType.Exp
    mul_op = mybir.AluOpType.mult
    add_op = mybir.AluOpType.add

    nc = bacc.Bacc("TRN2", target_bir_lowering=False, debug=False)
    qT_d = nc.dram_tensor("qT", [SIZE, S], bf16, kind="ExternalInput").ap()
    kT_d = nc.dram_tensor("kT", [SIZE, S], bf16, kind="ExternalInput").ap()
    vT_d = nc.dram_tensor("vT", [SIZE, S], bf16, kind="ExternalInput").ap()
    WqT_d = nc.dram_tensor("WqT", [SIZE, D_LOC], bf16, kind="ExternalInput").ap()
    WkT_d = nc.dram_tensor("WkT", [SIZE, D_LOC], bf16, kind="ExternalInput").ap()
    WvT_d = nc.dram_tensor("WvT", [SIZE, D_LOC], bf16, kind="ExternalInput").ap()
    WoT_d = nc.dram_tensor("WoT", [D_LOC, SIZE], bf16, kind="ExternalInput").ap()
    bq_d = nc.dram_tensor("bq", [D_LOC], f32, kind="ExternalInput").ap()
    bk_d = nc.dram_tensor("bk", [D_LOC], f32, kind="ExternalInput").ap()
    out_d = nc.dram_tensor("out", [S, SIZE], f32, kind="ExternalOutput").ap()

    qTt = qT_d.rearrange("(et p) s -> p et s", p=128)
    kTt = kT_d.rearrange("(et p) s -> p et s", p=128)
    vTt = vT_d.rearrange("(et p) s -> p et s", p=128)

    with tile.TileContext(nc) as tc:
        with tc.tile_pool(name="persist", bufs=1) as persist:
            wq_sb = persist.tile([128, ET, D_LOC], bf16)
            wk_sb = persist.tile([128, ET, D_LOC], bf16)
            wv_sb = persist.tile([128, ET, D_LOC], bf16)
            nc.sync.dma_start(wv_sb[:], WvT_d.rearrange("(et p) m -> p et m", p=128))
            nc.sync.dma_start(wq_sb[:], WqT_d.rearrange("(et p) m -> p et m", p=128))
            nc.sync.dma_start(wk_sb[:], WkT_d.rearrange("(et p) m -> p et m", p=128))
            wo_sb = persist.tile([128, MT, SIZE], bf16)
            nc.sync.dma_start(wo_sb[:], WoT_d.rearrange("(hp p) o -> p hp o", p=128))
            bq_sb = persist.tile([128, MT], f32)
            bk_sb = persist.tile([128, MT], f32)
            nc.sync.dma_start(bq_sb[:], bq_d.rearrange("(mt p) -> p mt", p=128))
            nc.sync.dma_start(bk_sb[:], bk_d.rearrange("(mt p) -> p mt", p=128))

            qh_sb = persist.tile([128, MT, S], bf16)   # [dim within pair, pair, s]
            kh_sb = persist.tile([128, MT, S], bf16)
            # [s%128, s//128, head, d] with a 65th ones-column per head: the
            # ctx matmul then accumulates sum(exp) in accumulator row 64.
            vh_sb = persist.tile([128, ST, H_LOC, D + 1], bf16)
            nc.vector.memset(vh_sb[:, :, :, D:D + 1], 1.0)
            ctxn_sb = persist.tile([128, MT, S], bf16)  # normalized ctxT

            # ---------- phase A: projections, one row-DMA per contraction tile ----------
            with (
                tc.tile_pool(name="xinV", bufs=4) as xinV,
                tc.tile_pool(name="psV", bufs=1, space="PSUM") as psV,
            ):
                # V first, in 2 passes of 8 sequence tiles (8 PSUM banks each)
                for vpass in range(2):
                    psv = [psV.tile([128, D_LOC], f32, tag=f"pv{i}", name=f"pv{i}")
                           for i in range(8)]
                    for et in range(ET):
                        vrow = xinV.tile([128, 1024], bf16, tag="vrow")
                        nc.sync.dma_start(
                            vrow[:], vTt[:, et, vpass * 1024:(vpass + 1) * 1024])
                        for i in range(8):
                            nc.tensor.matmul(
                                psv[i][:], vrow[:, i * 128:(i + 1) * 128], wv_sb[:, et, :],
                                start=(et == 0), stop=(et == ET - 1))
                    for i in range(8):
                        st = vpass * 8 + i
                        nc.vector.tensor_copy(
                            vh_sb[:, st, :, 0:D],
                            psv[i].rearrange("p (h d) -> p h d", h=H_LOC))
            with (
                tc.tile_pool(name="xinQK", bufs=4) as xinQK,
                tc.tile_pool(name="psQK", bufs=1, space="PSUM") as psQK,
            ):
                for src, wsb, bsb, dst in (
                    (kTt, wk_sb, bk_sb, kh_sb),
                    (qTt, wq_sb, bq_sb, qh_sb),
                ):
                    pss = [psQK.tile([128, 512], f32, tag=f"pa{i}", name=f"pa{i}")
                           for i in range(8)]
                    for et in range(ET):
                        xrow = xinQK.tile([128, S], bf16, tag="xrow")
                        nc.sync.dma_start(xrow[:, 0:1024], src[:, et, 0:1024])
                        nc.sync.dma_start(xrow[:, 1024:2048], src[:, et, 1024:2048])
                        # mt outer: one LDWEIGHTS serves 4 N=512 matmuls
                        for mt in range(MT):
                            for nt in range(4):
                                nc.tensor.matmul(
                                    pss[nt * MT + mt][:],
                                    wsb[:, et, mt * 128:(mt + 1) * 128],
                                    xrow[:, nt * 512:(nt + 1) * 512],
                                    start=(et == 0), stop=(et == ET - 1))
                    for nt in range(4):
                        for mt in range(MT):
                            nc.vector.tensor_scalar_add(
                                dst[:, mt, nt * 512:(nt + 1) * 512],
                                pss[nt * MT + mt][:], bsb[:, mt:mt + 1])

            # ---------- phase B: attention ----------
            with (
                tc.tile_pool(name="psS", bufs=1, space="PSUM") as psS,
                tc.tile_pool(name="psC", bufs=1, space="PSUM") as psC,
                tc.tile_pool(name="esb", bufs=2) as esb,
                tc.tile_pool(name="smalls", bufs=2) as smalls,
                tc.tile_pool(name="dscr", bufs=2, space="DRAM") as dscr,
            ):
                for pr in range(MT):
                    for sw in range(2):
                        q0s = sw * 1024
                        scs = [psS.tile([128, 1024], f32, tag=f"sc{hi}",
                                        name=f"sc{hi}")
                               for hi in range(2)]
                        # ctx accumulators [65,512]: rows 0-63 ctx, row 64 =
                        # sum(exp) via vh's ones column. Both heads at array
                        # col-strips 0-2 (serial); pair layout is restored by
                        # a SBUF->SBUF DMA after normalize.
                        cps = [[psC.tile([D + 1, 512], f32, tag=f"c{hi}{j}",
                                         name=f"c{hi}{j}")
                                for j in range(2)]
                               for hi in range(2)]
                        # single-writer exp tiles: j=0 half from ScalarE
                        # (exact), j=1 half from VectorE (Schraudolph int16)
                        eA = [[esb.tile([128, 512], bf16, tag=f"eA{hi}{a}",
                                        name=f"eA{hi}{a}") for a in range(2)]
                              for hi in range(2)]
                        eV = [[esb.tile([128, 512], bf16, tag=f"eV{hi}{a}",
                                        name=f"eV{hi}{a}") for a in range(2)]
                              for hi in range(2)]
                        for kt in range(KT):
                            # adjacent head pairs -> concurrent row-group streams
                            for j in range(2):
                                for hi in range(2):
                                    po = hi * D
                                    nc.tensor.matmul(
                                        scs[hi][:, j * 512:(j + 1) * 512],
                                        kh_sb[po:po + D, pr, kt * 128:(kt + 1) * 128],
                                        qh_sb[po:po + D, pr,
                                              q0s + j * 512:q0s + (j + 1) * 512],
                                        start=True, stop=True)
                            for hi in range(2):
                                ea = eA[hi][kt % 2]
                                ev = eV[hi][kt % 2]
                                nc.scalar.activation(
                                    ea[:], scs[hi][:, 0:512], Exp)
                                nc.vector.tensor_scalar(
                                    ev[:].bitcast(i16),
                                    scs[hi][:, 512:1024],
                                    EXP_A, EXP_B, mul_op, add_op)
                            st_, sp_ = (kt == 0), (kt == KT - 1)
                            for hi in range(2):
                                nc.tensor.matmul(
                                    cps[hi][0][:],
                                    vh_sb[:, kt, pr * 2 + hi, :],
                                    eA[hi][kt % 2][:],
                                    start=st_, stop=sp_)
                                nc.tensor.matmul(
                                    cps[hi][1][:],
                                    vh_sb[:, kt, pr * 2 + hi, :],
                                    eV[hi][kt % 2][:],
                                    start=st_, stop=sp_)
                        # evacuate ctx + denominators on ScalarE (frees banks)
                        cus = [smalls.tile([D + 1, 1024], f32, tag=f"cu{hi}", name=f"cu{hi}")
                               for hi in range(2)]
                        for hi in range(2):
                            for j in range(2):
                                nc.scalar.copy(
                                    cus[hi][:, j * 512:(j + 1) * 512],
                                    cps[hi][j][:])
                        scr = dscr.tile([2048], f32, tag="scr")
                        for hi in range(2):
                            for j in range(2):
                                hj = hi * 2 + j
                                nc.sync.dma_start(
                                    scr[hj * 512:(hj + 1) * 512]
                                    .rearrange("(a b) -> a b", a=1),
                                    cus[hi][D:D + 1, j * 512:(j + 1) * 512])
                        # reciprocal on a 64-partition spread, then broadcast
                        spread = smalls.tile([64, 32], f32, tag="spread")
                        nc.sync.dma_start(
                            spread[:], scr[:].rearrange("(p j) -> p j", p=64))
                        spread_r = smalls.tile([64, 32], f32, tag="spreadr")
                        nc.vector.reciprocal_approx_fast(spread_r[:], spread[:])
                        scr2 = dscr.tile([2048], f32, tag="scr2")
                        nc.sync.dma_start(
                            scr2[:].rearrange("(p j) -> p j", p=64), spread_r[:])
                        brecs = [smalls.tile([D, 1024], f32, tag=f"brec{hi}", name=f"brec{hi}")
                                 for hi in range(2)]
                        for hi in range(2):
                            for j in range(2):
                                hj = hi * 2 + j
                                part = scr2[hj * 512:(hj + 1) * 512]
                                nc.sync.dma_start(
                                    brecs[hi][:, j * 512:(j + 1) * 512],
                                    bass.AP(tensor=part.tensor, offset=part.offset,
                                            ap=[[0, D]] + list(part.ap)))
                        # normalize on GpSimd; h1 remapped to partitions
                        # 64-127 by a local DMA
                        nc.gpsimd.tensor_mul(
                            ctxn_sb[0:D, pr, q0s:q0s + 1024],
                            cus[0][0:D, :], brecs[0][:])
                        c1n = smalls.tile([D, 1024], bf16, tag="c1n", name="c1n")
                        nc.gpsimd.tensor_mul(
                            c1n[:], cus[1][0:D, :], brecs[1][:])
                        nc.sync.dma_start(
                            ctxn_sb[D:128, pr, q0s:q0s + 1024], c1n[:])

            # ---------- phase C: output projection (partial over local dims) ----------
            with (
                tc.tile_pool(name="psD", bufs=4, space="PSUM") as psD,
                tc.tile_pool(name="osb", bufs=4) as osb,
            ):
                for st in range(ST):
                    for ot in range(SIZE // 512):
                        pso = psD.tile([128, 512], f32, tag="po")
                        for hp in range(MT):
                            nc.tensor.matmul(
                                pso[:],
                                ctxn_sb[:, hp, st * 128:(st + 1) * 128],
                                wo_sb[:, hp, ot * 512:(ot + 1) * 512],
                                start=(hp == 0), stop=(hp == MT - 1))
                        o_sb = osb.tile([128, 512], f32, tag="o")
                        if (st * 2 + ot) % 2 == 0:
                            nc.vector.tensor_copy(o_sb[:], pso[:])
                        else:
                            nc.scalar.copy(o_sb[:], pso[:])
                        nc.sync.dma_start(
                            out_d[st * 128:(st + 1) * 128, ot * 512:(ot + 1) * 512],
                            o_sb[:])

    nc.compile()
    _NC = nc
    return nc


def prepare_in_maps(inputs):
    q, k, v = inputs["q"], inputs["k"], inputs["v"]
    Wq, bq = inputs["Wq"], inputs["bq"]
    Wk, bk = inputs["Wk"], inputs["bk"]
    Wv = inputs["Wv"]
    Wo = inputs["Wo"]
    sc = np.float32(1.0 / np.sqrt(D))

    f32, bf = np.float32, ml_dtypes.bfloat16
    qT = [q[b].T.astype(bf) for b in range(B)]
    kT = [k[b].T.astype(bf) for b in range(B)]
    vT = [v[b].T.astype(bf) for b in range(B)]
    WqTs = (Wq.T * sc).astype(bf)   # scale folded into Wq
    WkT = Wk.T.astype(bf)
    WvT = Wv.T.astype(bf)
    WoT = Wo.T.astype(bf)           # [c, o]
    bqs = (bq * sc).astype(f32)

    in_maps = []
    for core in range(NCORES):
        b, hg = divmod(core, HGROUPS)
        sl = slice(hg * D_LOC, (hg + 1) * D_LOC)
        in_maps.append({
            "qT": qT[b], "kT": kT[b], "vT": vT[b],
            "WqT": np.ascontiguousarray(WqTs[:, sl]),
            "WkT": np.ascontiguousarray(WkT[:, sl]),
            "WvT": np.ascontiguousarray(WvT[:, sl]),
            "WoT": np.ascontiguousarray(WoT[sl, :]),
            "bq": np.ascontiguousarray(bqs[sl]),
            "bk": np.ascontiguousarray(bk[sl].astype(f32)),
        })
    return in_maps


def gather(results, inputs):
    # host epilogue: sum the 4 tensor-parallel partials per batch and add the
    # constant row bv @ Wo.T + bo (the value bias commutes through softmax)
    const = (inputs["bv"].astype(np.float64) @ inputs["Wo"].astype(np.float64).T
             + inputs["bo"].astype(np.float64)).astype(np.float32)
    full = np.empty((B, S, SIZE), np.float32)
    for b in range(B):
        acc = results[b * HGROUPS]["out"].astype(np.float32).copy()
        for hg in range(1, HGROUPS):
            acc += results[b * HGROUPS + hg]["out"]
        full[b] = acc + const[None, :]
    return full


def kernel(**inputs):
    nc = build()
    in_maps = prepare_in_maps(inputs)
    res = run_bass_kernel_spmd(nc, in_maps, core_ids=list(range(NCORES)), trace=False)
    return gather(res.results, inputs)
